# revision 54
# baseline (speedup 1.0000x reference)
"""Trainium2 Bass kernel: linear attention (softmax over feature dim) MHA.

Math (per batch m, head h):
    q = softmax_d(xq @ Wq) * D**-0.5 ; k = softmax_d(xk @ Wk) ; v = xv @ Wv
    kv_h = k_h^T @ v_h            [d, e]
    out_h = q_h @ kv_h            [n, e]
    out = concat_h(out_h) @ Wo + bo

Sharding: data-parallel over batch m (16 batches -> 2 per core, 8 cores).
No collectives. Host-side marshalling: per-core shards are uploaded as
bf16, with x tensors pre-transposed to [batch, d_model, n] so every
matmul contraction sits on the SBUF partition axis.

Device pipeline per (core, batch):
  pass 1 (tokens in chunks of 128):
    psum_k = xkT^T @ Wk            (k in natural [tok, f] layout)
    ke     = exp(psum_k)           -> bf16 SBUF      (ACT)
    s_k    = rowsum per head       (DVE segmented reduce)
    rk     = 1/s_k                 (DVE)
    psum_v = xvT^T @ Wv
    vs     = psum_v * rk[bcast]    -> bf16 SBUF      (DVE, k-softmax folded into v)
    kvT_g += vs_pair^T @ ke_pair   (PE, two heads packed per 128x128 matmul;
                                    cross-head blocks discarded below)
  kvblkT_g = block-diag([kvT_2g, kvT_2g+1])  -> bf16 SBUF
  W2_g   = kvblkT_g^T @ Wo_pair    (merged-Wo: (qe@kv)@Wo == qe@(kv@Wo))
  pass 2 (tokens in chunks of 512, software-pipelined one chunk deep):
    psum_q = Wq^T @ xqT            (q in transposed [f, tok] layout)
    qe     = exp(psum_q)           -> bf16
    s_q   += pool8^T @ qe          (PE partition-pooling matmul)
    rq     = 1/s_q                 (DVE reciprocal_approx_fast, ~18-bit)
    bc     = exp8^T @ rq           (PE broadcast of rq across partitions)
    qen    = qe * bc               -> bf16 (q-softmax denominator applied)
    psum_o = qen^T @ W2            (output projection with kv pre-merged;
                                    D**-0.5 folded into Wo on host)
    out    = copy(psum_o) -> f32 -> DRAM
  The pipeline emits [recip(t); bc(t); qen(t); qproj(t+1)+pool(t+1);
  out(t)] so the PE streams qproj of chunk t+1 while the DVE normalizes
  chunk t.
PSUM: pq(2 banks: ps_k/ps_q) + po(2: ps_v/ps_o) + pkv(4: kv
accumulators, s, bc, w2) = 8 banks; the qproj/out split keeps the two
pass-2 matmul streams from serializing on a shared pool.
bo is added on the host (it is tiny); output returned as f32.
"""

import os
import sys

for _p in ("/opt/trn_rl_repo", "/root/.axon_site/_ro/trn_rl_repo"):
    if os.path.isdir(_p) and _p not in sys.path:
        sys.path.insert(0, _p)

from contextlib import ExitStack

import ml_dtypes
import numpy as np

import concourse.mybir as mybir
import concourse.tile as tile
from concourse import bacc, bass_isa
from concourse.bass import _add_dep_helper, ds, ts
from concourse.bass_utils import run_bass_kernel_spmd

BF16 = mybir.dt.bfloat16
F32 = mybir.dt.float32
F32R = mybir.dt.float32r
NPBF16 = ml_dtypes.bfloat16

M, N, DM = 16, 2048, 512
H, D = 8, 64
NCORES = 8
MB = M // NCORES          # batches per core
NC_DM = DM // 128         # 4 contraction chunks of 128
NT128 = N // 128          # 16 token chunks (pass 1)
NT512 = N // 512          # 4 token chunks (pass 2)
NPAIR = H // 2            # 4 head pairs

EXP = mybir.ActivationFunctionType.Exp
COPY = mybir.ActivationFunctionType.Copy


def build_program(reps: int = 1, loop_n: int = 1):
    nc = bacc.Bacc(
        "TRN2", target_bir_lowering=False, debug=False, num_devices=NCORES
    )
    # x and w are uploaded pre-shuffled so each DMA descriptor covers >=4KB
    # of contiguous memory on BOTH sides (1-2KB lines cap the DMA engines
    # well below HBM rate). x layout: [batch, partition, piece, chunk, 512]
    # -- token pieces are major so a piece load is contiguous per partition.
    xqT = nc.dram_tensor("xqT", [MB, 128, 4, NC_DM, 512], BF16, kind="ExternalInput").ap()
    xkT = nc.dram_tensor("xkT", [MB, 128, 4, NC_DM, 512], BF16, kind="ExternalInput").ap()
    xvT = nc.dram_tensor("xvT", [MB, 128, 4, NC_DM, 512], BF16, kind="ExternalInput").ap()
    w_dram = {
        name: nc.dram_tensor(name, [128, NC_DM, DM], BF16, kind="ExternalInput").ap()
        for name in ("wq", "wk", "wv", "wo")
    }
    # pool8[p, c, 32r+h] = 1 iff h == 2c + p//64 : per-head partition
    # pooling, with the head sums replicated at partition offsets 0/32/64/96
    # so the four bc broadcast matmuls can run on distinct PE row groups
    pool8_d = nc.dram_tensor("pool8", [128, NC_DM, 128], BF16, kind="ExternalInput").ap()
    # exp8[32c+h, j] = 1 iff h == 2c + j//64 : partition broadcast for chunk c
    exp8_d = nc.dram_tensor("exp8", [128, 128], BF16, kind="ExternalInput").ap()
    out_d = nc.dram_tensor("out", [MB, N, DM], F32, kind="ExternalOutput").ap()

    with tile.TileContext(nc) as tc, ExitStack() as ctx:
        wpool = ctx.enter_context(tc.tile_pool(name="w", bufs=1))
        xpool = ctx.enter_context(tc.tile_pool(name="x", bufs=2))
        kepool = ctx.enter_context(tc.tile_pool(name="ke", bufs=6))
        vspool = ctx.enter_context(tc.tile_pool(name="vs", bufs=6))
        skpool = ctx.enter_context(tc.tile_pool(name="sk", bufs=8))
        kbpool = ctx.enter_context(tc.tile_pool(name="kvblk", bufs=8))
        qepool = ctx.enter_context(tc.tile_pool(name="qe", bufs=10))
        rqpool = ctx.enter_context(tc.tile_pool(name="rq", bufs=4))
        o5pool = ctx.enter_context(tc.tile_pool(name="o5", bufs=10))
        w2pool = ctx.enter_context(tc.tile_pool(name="w2", bufs=8))
        fpool = ctx.enter_context(tc.tile_pool(name="fin", bufs=4))
        ps_q = ctx.enter_context(tc.tile_pool(name="ps_q", bufs=2, space="PSUM"))
        ps_o = ctx.enter_context(tc.tile_pool(name="ps_o", bufs=2, space="PSUM"))
        ps_kv = ctx.enter_context(tc.tile_pool(name="ps_kv", bufs=4, space="PSUM"))

        # weights in consumption order: wk/wv gate the first matmuls.
        # For the single-shot program, wq/wo loads are deferred until after
        # batch 0's x loads so they don't delay the first kproj matmul.
        defer_qo = reps == 1 and loop_n == 1
        w_sb = {
            name: wpool.tile([128, NC_DM, DM], BF16, tag=name, name=name)
            for name in ("wk", "wv", "wq", "wo")
        }

        # In the single-shot program the head is DMA-critical: the Sync
        # stream is ordered wk, xk-piece0, wv, xv-piece0, <rest>, and the
        # pass-2 weights (~1MB) load via the ACT engine's DGE queue with a
        # dependency edge anchoring them behind the first exp so they can't
        # steal DMA bandwidth from the head.
        def load_w(name, eng):
            return eng.dma_start(out=w_sb[name][:, :, :], in_=w_dram[name])

        pool8_sb = wpool.tile([128, NC_DM, 128], BF16, tag="pool8")
        exp8_sb = wpool.tile([128, 128], BF16, tag="exp8")

        def load_pass2_consts():
            return [
                load_w("wq", nc.scalar),
                load_w("wo", nc.scalar),
                nc.scalar.dma_start(out=pool8_sb[:, :, :], in_=pool8_d),
                nc.scalar.dma_start(out=exp8_sb[:, :], in_=exp8_d),
            ]

        if not defer_qo:
            load_w("wk", nc.sync)
            load_w("wv", nc.sync)
            load_pass2_consts()

        loop_ctx = (
            tc.For_i(
                0, loop_n, 1,
                hint_engines=tuple(mybir.EngineType) if loop_n > 1 else (),
            )
            if loop_n > 1
            else None
        )
        if loop_ctx is not None:
            ctx.enter_context(loop_ctx)
        for _rep in range(reps):
            for b in range(MB):
                # xk/xv land in 512-token pieces so the first kproj/vproj
                # only wait on one piece; DMA engines drain descriptors
                # FIFO, so early pieces finish first at full rate. xq
                # (pass 2 only) loads whole.
                xk_sb = xpool.tile([128, 4, NC_DM, 512], BF16, tag="xk")
                xv_sb = xpool.tile([128, 4, NC_DM, 512], BF16, tag="xv")
                xq_sb = xpool.tile([128, 4, NC_DM, 512], BF16, tag="xq")
                for p in range(4):
                    for src, dst, wname in (
                        (xkT, xk_sb, "xk"),
                        (xvT, xv_sb, "wv"),
                    ):
                        if defer_qo and b == 0 and p == 0:
                            if wname == "wv":
                                load_w("wv", nc.sync)
                            else:
                                load_w("wk", nc.sync)
                            # piece 0 lands in two 256-token halves so the
                            # first projection matmuls unlock ~1us earlier
                            for hh in range(2):
                                nc.sync.dma_start(
                                    out=dst[:, 0, :, ds(256 * hh, 256)],
                                    in_=src[b][:, 0, :, ds(256 * hh, 256)],
                                )
                        else:
                            nc.sync.dma_start(
                                out=dst[:, p, :, :],
                                in_=src[b][:, p, :, :],
                            )
                nc.sync.dma_start(out=xq_sb[:, :, :, :], in_=xqT[b])

                # ---------------- pass 1: kv = k_sm^T v --------------------
                kv_ps = [
                    ps_kv.tile([128, 128], F32, tag="kv", name=f"kv{g}")
                    for g in range(NPAIR)
                ]
                for tk in range(NT128):
                    ps_k = ps_q.tile([128, DM], F32, tag="q")
                    for c in range(NC_DM):
                        nc.tensor.matmul(
                            ps_k[:, :],
                            xk_sb[:, tk // 4, c, ds(128 * (tk % 4), 128)],
                            w_sb["wk"][:, c, :],
                            start=(c == 0),
                            stop=(c == NC_DM - 1),
                        )
                    ke = kepool.tile([128, H, D], BF16, tag="ke")
                    ke_act = nc.scalar.activation(
                        ke[:, :, :],
                        ps_k[:, :].rearrange("p (h e) -> p h e", h=H),
                        EXP,
                    )
                    if defer_qo and b == 0 and tk == 0:
                        for di in load_pass2_consts():
                            _add_dep_helper(
                                di.ins, ke_act.ins, sync=False,
                                reason="hold const DMAs off the critical head",
                            )
                    sk = skpool.tile([128, H], F32, tag="sk")
                    nc.vector.tensor_reduce(
                        sk[:, :],
                        ke[:, :, :],
                        axis=mybir.AxisListType.X,
                        op=mybir.AluOpType.add,
                    )
                    rk = skpool.tile([128, H], F32, tag="rk")
                    nc.vector.reciprocal(rk[:, :], sk[:, :])

                    ps_v = ps_o.tile([128, DM], F32, tag="o")
                    for c in range(NC_DM):
                        nc.tensor.matmul(
                            ps_v[:, :],
                            xv_sb[:, tk // 4, c, ds(128 * (tk % 4), 128)],
                            w_sb["wv"][:, c, :],
                            start=(c == 0),
                            stop=(c == NC_DM - 1),
                        )
                    vs = vspool.tile([128, H, D], BF16, tag="vs")
                    nc.vector.tensor_mul(
                        vs[:, :, :],
                        ps_v[:, :].rearrange("p (h e) -> p h e", h=H),
                        rk[:, :].to_broadcast([128, H, D]),
                    )
                    for g in range(NPAIR):
                        # kvT = vs^T ke, 2-head pack: stat/mov [128, 128];
                        # off-diagonal cross-head blocks are garbage, dropped.
                        nc.tensor.matmul(
                            kv_ps[g][:, :],
                            vs[:, ds(2 * g, 2), :],
                            ke[:, ds(2 * g, 2), :],
                            start=(tk == 0),
                            stop=(tk == NT128 - 1),
                        )

                w2s = []
                for g in range(NPAIR):
                    kb = kbpool.tile([128, 128], BF16, tag="kvblk")
                    nc.vector.memset(kb[:, :], 0.0)
                    nc.vector.tensor_copy(kb[0:64, 0:64], kv_ps[g][0:64, 0:64])
                    nc.vector.tensor_copy(kb[64:128, 64:128], kv_ps[g][64:128, 64:128])
                    w2_ps = ps_kv.tile([128, DM], F32, tag="kv", name="w2_ps")
                    nc.tensor.matmul(
                        w2_ps[:, :], kb[:, :], w_sb["wo"][:, g, :],
                        start=True, stop=True,
                    )
                    w2 = w2pool.tile([128, DM], BF16, tag="w2")
                    nc.scalar.activation(w2[:, :], w2_ps[:, :], COPY)
                    w2s.append(w2)

                # ---------------- pass 2: out = (q_sm @ kv) @ Wo -----------
                # software pipeline: qproj/pool of chunk t+1 is emitted
                # between the DVE normalization and the out matmuls of
                # chunk t, so the PE never drains while the DVE catches up.
                def qproj_pool(t):
                    s_ps = ps_kv.tile([128, 512], F32, tag="kv", name="s_ps")
                    qes = []
                    for c in range(NC_DM):
                        psq = ps_q.tile([128, 512], F32, tag="q")
                        for k in range(NC_DM):
                            nc.tensor.matmul(
                                psq[:, :],
                                w_sb["wq"][:, k, ds(128 * c, 128)],
                                xq_sb[:, t, k, :],
                                start=(k == 0),
                                stop=(k == NC_DM - 1),
                            )
                        qe = qepool.tile([128, 512], BF16, tag="qe")
                        nc.scalar.activation(qe[:, :], psq[:, :], EXP)
                        nc.tensor.matmul(
                            s_ps[:, :],
                            pool8_sb[:, c, :],
                            qe[:, :],
                            start=(c == 0),
                            stop=(c == NC_DM - 1),
                        )
                        qes.append(qe)
                    return s_ps, qes

                s_cur, qes_cur = qproj_pool(0)
                for t in range(NT512):
                    # s rows 32r+h hold the head-h sum (others are 0 -> inf
                    # after reciprocal, never read by the bc stationaries)
                    rq32 = rqpool.tile([128, 512], F32, tag="rq32")
                    nc.vector.reciprocal_approx_fast(rq32[:, :], s_cur[:, :])
                    rq = rqpool.tile([128, 512], BF16, tag="rq")
                    nc.vector.tensor_copy(rq[:, :], rq32[:, :])

                    bcs = []
                    for c in range(NC_DM):
                        # K=8 stationaries on distinct PE row groups: the 4
                        # bc matmuls execute concurrently (~one 512-col span)
                        bc = ps_kv.tile([128, 512], F32, tag="kv", name="bc")
                        nc.tensor.matmul(
                            bc[:, :],
                            exp8_sb[ds(32 * c, 8), :],
                            rq[ds(32 * c, 8), :],
                            start=True, stop=True,
                            tile_position=(32 * c, 0),
                        )
                        bcs.append(bc)
                    qens = []
                    for c in range(NC_DM):
                        qen = o5pool.tile([128, 512], BF16, tag="qen")
                        nc.vector.tensor_mul(qen[:, :], qes_cur[c][:, :], bcs[c][:, :])
                        qens.append(qen)

                    if t + 1 < NT512:
                        s_cur, qes_cur = qproj_pool(t + 1)

                    for u in range(4):
                        pso = ps_o.tile([128, DM], F32, tag="o")
                        for c in range(NC_DM):
                            nc.tensor.matmul(
                                pso[:, :],
                                qens[c][:, ds(128 * u, 128)],
                                w2s[c][:, :],
                                start=(c == 0),
                                stop=(c == NC_DM - 1),
                            )
                        fin = fpool.tile([128, DM], F32, tag="fin")
                        nc.scalar.activation(fin[:, :], pso[:, :], COPY)
                        nc.sync.dma_start(
                            out=out_d[b, ds(512 * t + 128 * u, 128), :],
                            in_=fin[:, :],
                        )
    nc.compile()
    return nc


def make_const_inputs():
    pool8 = np.zeros((128, NC_DM, 128), np.float32)
    for p in range(128):
        for c in range(NC_DM):
            for r in range(4):
                pool8[p, c, 32 * r + 2 * c + p // 64] = 1.0
    exp8 = np.zeros((128, 128), np.float32)
    for c in range(NC_DM):
        for j in range(128):
            exp8[32 * c + 2 * c + j // 64, j] = 1.0
    return pool8.astype(NPBF16), exp8.astype(NPBF16)


def make_in_maps(xq, xk, xv, Wq, Wk, Wv, Wo):
    pool8, exp8 = make_const_inputs()
    scale = np.float32(D**-0.5)

    def prep_w(W, s=1.0):
        # [DM, DM] -> [128, NC_DM, DM] with w[p, c, f] = W[128c+p, f]
        w = (np.asarray(W, np.float32) * s).reshape(NC_DM, 128, DM)
        return np.ascontiguousarray(w.transpose(1, 0, 2)).astype(NPBF16)

    consts = {
        "wq": prep_w(Wq),
        "wk": prep_w(Wk),
        "wv": prep_w(Wv),
        "wo": prep_w(Wo, scale),
        "pool8": pool8,
        "exp8": exp8,
    }

    def prep(x, sl):
        # [mb, n, dm] -> [mb, 128, piece, NC_DM, 512]
        # with x[b, p, t, c, n'] = x[b, 512t + n', 128c + p]
        xt = np.asarray(x[sl], np.float32).transpose(0, 2, 1)
        xt = xt.reshape(MB, NC_DM, 128, 4, 512).transpose(0, 2, 3, 1, 4)
        return np.ascontiguousarray(xt).astype(NPBF16)

    in_maps = []
    for core in range(NCORES):
        sl = slice(MB * core, MB * (core + 1))
        m = dict(consts)
        m["xqT"] = prep(xq, sl)
        m["xkT"] = prep(xk, sl)
        m["xvT"] = prep(xv, sl)
        in_maps.append(m)
    return in_maps


_NC = None


def kernel(xq, xk, xv, Wq, Wk, Wv, Wo, bo):
    global _NC
    if _NC is None:
        _NC = build_program()
    in_maps = make_in_maps(xq, xk, xv, Wq, Wk, Wv, Wo)
    res = run_bass_kernel_spmd(_NC, in_maps, core_ids=list(range(NCORES)))
    out = np.concatenate([res.results[i]["out"] for i in range(NCORES)], axis=0)
    out += np.asarray(bo, np.float32)[None, None, :]
    return out
